# revision 8
# baseline (speedup 1.0000x reference)
"""Channel self-attention kernel for Trainium2 (8 NeuronCores, data-parallel over batch).

Per sample (x: [C=512, N=1024], qm/km: [N, 64], W: [C, C], b: [C]):
  q = x @ qm ; k = x @ km                  (contract over n -> needs x^T)
  energy = q @ k^T                         [C, C]
  E = exp(sigmoid(energy))                 (sigmoid via tanh: one ACT table set)
  att = E / colsum(E)                      (softmax over rows, axis -2)
  v = W @ x + b
  out[j, n] = sum_i att[i, j] v[i, n]
Reassociated as:
  M1T[c', j] = sum_i W[i, c'] E[i, j]      (native layouts, no PE transposes)
  unnorm[j, n] = sum_c' M1T[c', j] x[c', n]
  out = unnorm * recip[j] + (E^T b)[j] * recip[j]

Numerics: the host splits x and [qm|km] into fp16 hi/mid pairs (x = hi + mid
to ~21 mantissa bits). fp16 matmuls run at 1 PE cycle/row with exact products
accumulated in fp32 PSUM, and fp16 is XBAR-eligible so x^T comes from
DMA-transpose instead of the (lossy, 12-bit) PE transpose path:
  q/k   : 3-term hi/lo matmuls        -> ~2^-17 relative accuracy
  energy: q/k re-split on device, 2 stacked K=128 matmuls per block
  value : single-term fp16 (contributes ~1e-4 of output absmax)
"""

import numpy as np

import concourse.bass as bass
import concourse.tile as tile
from concourse import bacc, mybir
from concourse.bass_utils import run_bass_kernel_spmd

B, C, N, D = 64, 512, 1024, 64
H = W_SP = 32
NCORES = 8
BS = B // NCORES  # samples per core
P = 128
CCH = C // P  # 4 channel chunks
NCH = N // P  # 8 n chunks

F32 = mybir.dt.float32
F16 = mybir.dt.float16


def _emit(tc, xh_d, xm_d, qkmh_d, qkmm_d, w_d, b_d, o_d):
    nc = tc.nc
    from concourse.masks import make_identity
    Tanh = mybir.ActivationFunctionType.Tanh
    Exp = mybir.ActivationFunctionType.Exp
    Identity = mybir.ActivationFunctionType.Identity

    with (
        tc.tile_pool(name="consts", bufs=1) as consts,
        tc.tile_pool(name="xh", bufs=3) as xh_pool,
        tc.tile_pool(name="xfT", bufs=2) as xfT_pool,
        tc.tile_pool(name="tE", bufs=2) as tE_pool,
        tc.tile_pool(name="m1t", bufs=2) as m1t_pool,
        tc.tile_pool(name="small", bufs=3) as small_pool,
        tc.tile_pool(name="outp", bufs=4) as out_pool,
        tc.tile_pool(name="ps", bufs=8, space="PSUM") as ps,
    ):
        ident = consts.tile([P, P], F32, name="ident", tag="ident")
        make_identity(nc, ident[:])

        qkmh_sb = consts.tile([P, NCH, P], F16, name="qkmh_sb", tag="qkmh_sb")
        nc.sync.dma_start(qkmh_sb[:], qkmh_d.ap().rearrange("(o p) d -> p o d", p=P))
        qkmm_sb = consts.tile([P, NCH, P], F16, name="qkmm_sb", tag="qkmm_sb")
        nc.sync.dma_start(qkmm_sb[:], qkmm_d.ap().rearrange("(o p) d -> p o d", p=P))
        w_sb = consts.tile([P, CCH, C], F16, name="w_sb", tag="w_sb")
        nc.sync.dma_start(w_sb[:], w_d.ap().rearrange("(o p) c -> p o c", p=P))

        # ob_sb[:, icc, 0] = 1.0 (denominator), ob_sb[:, icc, 1] = b[icc*128+p]
        b_tmp = consts.tile([P, CCH], F32, name="b_tmp", tag="b_tmp")
        nc.sync.dma_start(b_tmp[:], b_d.ap().rearrange("(o p) -> p o", p=P))
        ob_sb = consts.tile([P, CCH, 2], F16, name="ob_sb", tag="ob_sb")
        nc.gpsimd.memset(ob_sb[:], 1.0)
        nc.vector.tensor_copy(ob_sb[:, :, 1], b_tmp[:])

        half = consts.tile([P, 1], F32, name="half", tag="half")
        nc.gpsimd.memset(half[:], 0.5)

        # Energy operand tiles (persistent): rows 0:64 / 64:128 are rewritten
        # per sample; TQ2/TK2 rows 64:128 stay zero so the second energy
        # matmul contributes only q_hi * k_mid.
        TQ1 = consts.tile([P, C], F16, name="TQ1", tag="TQ1")
        TK1 = consts.tile([P, C], F16, name="TK1", tag="TK1")
        TQ2 = consts.tile([P, C], F16, name="TQ2", tag="TQ2")
        TK2 = consts.tile([P, C], F16, name="TK2", tag="TK2")
        for t in (TQ1, TK1, TQ2, TK2):
            nc.gpsimd.memset(t[:], 0.0)

        evac_ctr = 0

        def evac(dst, src):
            # Alternate PSUM->SBUF evacuations between DVE and ACT.
            nonlocal evac_ctr
            if evac_ctr % 2 == 0:
                nc.vector.tensor_copy(dst, src)
            else:
                nc.scalar.copy(dst, src)
            evac_ctr += 1

        def stage_a(s):
            """Load x_s (straight + transposed), q/k projections, energy, E."""
            xh = xh_pool.tile([P, CCH, N], F16, name="xh", tag="xh")
            nc.sync.dma_start(xh[:], xh_d.ap()[s].rearrange("(o p) n -> p o n", p=P))

            xfTh = xfT_pool.tile([P, NCH, C], F16, name="xfTh", tag="xfTh")
            xfTm = xfT_pool.tile([P, NCH, C], F16, name="xfTm", tag="xfTm")
            for nb in range(NCH):
                nc.sync.dma_start_transpose(
                    xfTh[:, nb, :], xh_d.ap()[s][:, nb * P:(nb + 1) * P])
                nc.sync.dma_start_transpose(
                    xfTm[:, nb, :], xm_d.ap()[s][:, nb * P:(nb + 1) * P])

            # qk^T[d|dk, c] in one PSUM tile: rows 0:64 = q^T, 64:128 = k^T.
            psqk = ps.tile([P, 512], F32, name="psqk", tag="ps")
            nmm = 3 * NCH
            i = 0
            for nb in range(NCH):
                for lhsT, rhs in (
                    (qkmh_sb[:, nb, :], xfTh[:, nb, :]),
                    (qkmm_sb[:, nb, :], xfTh[:, nb, :]),
                    (qkmh_sb[:, nb, :], xfTm[:, nb, :]),
                ):
                    nc.tensor.matmul(psqk[:], lhsT, rhs,
                                     start=(i == 0), stop=(i == nmm - 1))
                    i += 1

            qk_hi = small_pool.tile([P, 512], F16, name="qk_hi", tag="qk_hi")
            nc.vector.tensor_copy(qk_hi[:], psqk[:])
            qk_mid = small_pool.tile([P, 512], F16, name="qk_mid", tag="qk_mid")
            nc.vector.tensor_tensor(
                qk_mid[:], psqk[:], qk_hi[:], mybir.AluOpType.subtract)

            nc.vector.tensor_copy(TQ1[0:64, :], qk_hi[0:64, :])
            nc.vector.tensor_copy(TQ1[64:128, :], qk_mid[0:64, :])
            nc.vector.tensor_copy(TK1[0:64, :], qk_hi[64:128, :])
            nc.vector.tensor_copy(TK1[64:128, :], qk_hi[64:128, :])
            nc.vector.tensor_copy(TQ2[0:64, :], qk_hi[0:64, :])
            nc.vector.tensor_copy(TK2[0:64, :], qk_mid[64:128, :])

            tE = tE_pool.tile([P, CCH, C], F32, name="tE", tag="tE")
            E = tE_pool.tile([P, CCH, C], F16, name="E", tag="E")
            for cb in range(CCH):
                pse = ps.tile([P, 512], F32, name="pse", tag="ps")
                nc.tensor.matmul(pse[:], TQ1[:, cb * P:(cb + 1) * P], TK1[:],
                                 start=True, stop=False)
                nc.tensor.matmul(pse[:], TQ2[:, cb * P:(cb + 1) * P], TK2[:],
                                 start=False, stop=True)
                # sigmoid(x) = 0.5*tanh(0.5x) + 0.5 ; E = exp(0.5*t + 0.5).
                # tanh and exp share one ACT table set (exp_and_others).
                nc.scalar.activation(tE[:, cb, :], pse[:], Tanh, scale=0.5)
            for cb in range(CCH):
                nc.scalar.activation(
                    E[:, cb, :], tE[:, cb, :], Exp, bias=half[:, 0:1], scale=0.5,
                )
            return xh, E

        def stage_b(s, xh, E):
            """Denominators, M1T, final matmul + normalize + store."""
            # rows: [denom; E^T b] = [2, C]
            psr = ps.tile([2, 512], F32, name="psr", tag="ps")
            for icc in range(CCH):
                nc.tensor.matmul(
                    psr[:], ob_sb[:, icc, :], E[:, icc, :],
                    start=(icc == 0), stop=(icc == CCH - 1),
                )
            rows2 = small_pool.tile([2, C], F32, name="rows2", tag="rows2")
            nc.vector.tensor_copy(rows2[:], psr[:])
            nc.vector.reciprocal(rows2[0:1, :], rows2[0:1, :])
            # Transpose [2, C] rows into per-partition columns [128, 2] per j-block.
            psc = ps.tile([P, 2 * CCH], F32, name="psc", tag="ps")
            for jb in range(CCH):
                nc.tensor.transpose(
                    psc[:, 2 * jb:2 * jb + 2],
                    rows2[:, jb * P:(jb + 1) * P],
                    ident[0:2, 0:2],
                )
            scales = small_pool.tile([P, CCH, 2], F32, name="scales", tag="scales")
            nc.vector.tensor_copy(scales[:], psc[:])
            # scales[:, jb, 1] = eb * recip
            nc.vector.tensor_mul(scales[:, :, 1], scales[:, :, 1], scales[:, :, 0])

            # M1T[c', j] = sum_i W[i, c'] E[i, j]
            m1t = m1t_pool.tile([P, CCH, C], F16, name="m1t", tag="m1t")
            for cb in range(CCH):
                psm = ps.tile([P, 512], F32, name="psm", tag="ps")
                for icc in range(CCH):
                    nc.tensor.matmul(
                        psm[:], w_sb[:, icc, cb * P:(cb + 1) * P], E[:, icc, :],
                        start=(icc == 0), stop=(icc == CCH - 1),
                    )
                evac(m1t[:, cb, :], psm[:])

            # out[j, n] = (sum_c' M1T[c', j] x[c', n]) * recip[j] + eb*recip[j]
            o_view = o_d.ap()[s].rearrange("(o p) n -> p o n", p=P)
            for jb in range(CCH):
                pso = [
                    ps.tile([P, 512], F32, name=f"pso{h}", tag="ps")
                    for h in range(2)
                ]
                for cc in range(CCH):
                    for h in range(2):
                        nc.tensor.matmul(
                            pso[h][:],
                            m1t[:, cc, jb * P:(jb + 1) * P],
                            xh[:, cc, h * 512:(h + 1) * 512],
                            start=(cc == 0), stop=(cc == CCH - 1),
                        )
                for h in range(2):
                    ot = out_pool.tile([P, 512], F32, name="ot", tag="ot")
                    if (jb * 2 + h) % 2 == 0:
                        nc.vector.tensor_scalar(
                            ot[:], pso[h][:],
                            scales[:, jb, 0:1], scales[:, jb, 1:2],
                            mybir.AluOpType.mult, mybir.AluOpType.add,
                        )
                    else:
                        nc.scalar.activation(
                            ot[:], pso[h][:], Identity,
                            bias=scales[:, jb, 1:2], scale=scales[:, jb, 0:1],
                        )
                    nc.sync.dma_start(o_view[:, jb, h * 512:(h + 1) * 512], ot[:])

        prev = None
        for s in range(BS):
            cur = stage_a(s)
            if prev is not None:
                stage_b(s - 1, *prev)
            prev = cur
        stage_b(BS - 1, *prev)


_CACHE = {}


def _build():
    if "nc" in _CACHE:
        return _CACHE["nc"]
    nc = bacc.Bacc("TRN2", target_bir_lowering=False, debug=False,
                   num_devices=NCORES)
    xh_d = nc.dram_tensor("x_hi", [BS, C, N], F16, kind="ExternalInput")
    xm_d = nc.dram_tensor("x_mid", [BS, C, N], F16, kind="ExternalInput")
    qkmh_d = nc.dram_tensor("qkm_hi", [N, 2 * D], F16, kind="ExternalInput")
    qkmm_d = nc.dram_tensor("qkm_mid", [N, 2 * D], F16, kind="ExternalInput")
    w_d = nc.dram_tensor("value_w", [C, C], F16, kind="ExternalInput")
    b_d = nc.dram_tensor("value_b", [C], F32, kind="ExternalInput")
    o_d = nc.dram_tensor("out", [BS, C, N], F32, kind="ExternalOutput")
    with tile.TileContext(nc) as tc:
        _emit(tc, xh_d, xm_d, qkmh_d, qkmm_d, w_d, b_d, o_d)
    nc.compile()
    _CACHE["nc"] = nc
    return nc


def make_in_maps(x, query_m, key_m, value_w, value_b):
    x = np.ascontiguousarray(x.reshape(B, C, N), dtype=np.float32)
    x_hi = x.astype(np.float16)
    x_mid = (x - x_hi.astype(np.float32)).astype(np.float16)
    qkm = np.concatenate(
        [np.asarray(query_m, np.float32), np.asarray(key_m, np.float32)], axis=1
    )
    qkm_hi = qkm.astype(np.float16)
    qkm_mid = (qkm - qkm_hi.astype(np.float32)).astype(np.float16)
    common = {
        "qkm_hi": np.ascontiguousarray(qkm_hi),
        "qkm_mid": np.ascontiguousarray(qkm_mid),
        "value_w": np.ascontiguousarray(np.asarray(value_w).astype(np.float16)),
        "value_b": np.ascontiguousarray(value_b, dtype=np.float32),
    }
    return [
        {
            "x_hi": np.ascontiguousarray(x_hi[c * BS:(c + 1) * BS]),
            "x_mid": np.ascontiguousarray(x_mid[c * BS:(c + 1) * BS]),
            **common,
        }
        for c in range(NCORES)
    ]


def kernel(x, query_m, key_m, value_w, value_b):
    nc = _build()
    in_maps = make_in_maps(x, query_m, key_m, value_w, value_b)
    res = run_bass_kernel_spmd(nc, in_maps, list(range(NCORES)))
    out = np.concatenate([res.results[c]["out"] for c in range(NCORES)], axis=0)
    return out.reshape(B, C, H, W_SP).astype(np.float32)


# revision 9
# speedup vs baseline: 1.2669x; 1.2669x over previous
"""Channel self-attention kernel for Trainium2 (8 NeuronCores, data-parallel over batch).

Per sample (x: [C=512, N=1024], qm/km: [N, 64], W: [C, C], b: [C]):
  q = x @ qm ; k = x @ km                  (contract over n -> needs x^T)
  energy = q @ k^T                         [C, C]
  E = exp(sigmoid(energy))                 (sigmoid via tanh: one ACT table set)
  att = E / colsum(E)                      (softmax over rows, axis -2)
  v = W @ x + b
  out[j, n] = sum_i att[i, j] v[i, n]
Reassociated as:
  M1T[c', j] = sum_i W[i, c'] E[i, j]      (native layouts, no PE transposes)
  unnorm[j, n] = sum_c' M1T[c', j] x[c', n]
  out = unnorm * recip[j] + (E^T b)[j] * recip[j]

Numerics: the host splits x and [qm|km] into fp16 hi/mid pairs (x = hi + mid
to ~21 mantissa bits). fp16 matmuls run at 1 PE cycle/row with exact products
accumulated in fp32 PSUM, and fp16 is XBAR-eligible so x^T comes from
DMA-transpose instead of the (lossy, 12-bit) PE transpose path:
  q/k   : 3-term hi/lo matmuls        -> ~2^-17 relative accuracy
  energy: q/k re-split on device, 2 stacked K=128 matmuls per block
  value : single-term fp16 (contributes ~1e-4 of output absmax)
"""

import numpy as np

import concourse.bass as bass
import concourse.tile as tile
from concourse import bacc, mybir
from concourse.bass_utils import run_bass_kernel_spmd

B, C, N, D = 64, 512, 1024, 64
H = W_SP = 32
NCORES = 8
BS = B // NCORES  # samples per core
P = 128
CCH = C // P  # 4 channel chunks
NCH = N // P  # 8 n chunks

F32 = mybir.dt.float32
F16 = mybir.dt.float16


def _emit(tc, xh_d, xth_d, xtm_d, qkmh_d, qkmm_d, w_d, b_d, o_d):
    nc = tc.nc
    from concourse.masks import make_identity
    Tanh = mybir.ActivationFunctionType.Tanh
    Exp = mybir.ActivationFunctionType.Exp
    Identity = mybir.ActivationFunctionType.Identity

    with (
        tc.tile_pool(name="consts", bufs=1) as consts,
        tc.tile_pool(name="xh", bufs=3) as xh_pool,
        tc.tile_pool(name="xfT", bufs=2) as xfT_pool,
        tc.tile_pool(name="tE", bufs=2) as tE_pool,
        tc.tile_pool(name="m1t", bufs=2) as m1t_pool,
        tc.tile_pool(name="small", bufs=3) as small_pool,
        tc.tile_pool(name="outp", bufs=4) as out_pool,
        tc.tile_pool(name="ps", bufs=8, space="PSUM") as ps,
    ):
        ident = consts.tile([P, P], F32, name="ident", tag="ident")
        make_identity(nc, ident[:])

        qkmh_sb = consts.tile([P, NCH, P], F16, name="qkmh_sb", tag="qkmh_sb")
        nc.sync.dma_start(qkmh_sb[:], qkmh_d.ap().rearrange("(o p) d -> p o d", p=P))
        qkmm_sb = consts.tile([P, NCH, P], F16, name="qkmm_sb", tag="qkmm_sb")
        nc.sync.dma_start(qkmm_sb[:], qkmm_d.ap().rearrange("(o p) d -> p o d", p=P))
        w_sb = consts.tile([P, CCH, C], F16, name="w_sb", tag="w_sb")
        nc.sync.dma_start(w_sb[:], w_d.ap().rearrange("(o p) c -> p o c", p=P))

        # ob_sb[:, icc, 0] = 1.0 (denominator), ob_sb[:, icc, 1] = b[icc*128+p]
        b_tmp = consts.tile([P, CCH], F32, name="b_tmp", tag="b_tmp")
        nc.sync.dma_start(b_tmp[:], b_d.ap().rearrange("(o p) -> p o", p=P))
        ob_sb = consts.tile([P, CCH, 2], F16, name="ob_sb", tag="ob_sb")
        nc.gpsimd.memset(ob_sb[:], 1.0)
        nc.vector.tensor_copy(ob_sb[:, :, 1], b_tmp[:])

        half = consts.tile([P, 1], F32, name="half", tag="half")
        nc.gpsimd.memset(half[:], 0.5)

        # Energy operand tiles (persistent): rows 0:64 / 64:128 are rewritten
        # per sample; TQ2/TK2 rows 64:128 stay zero so the second energy
        # matmul contributes only q_hi * k_mid.
        TQ1 = consts.tile([P, C], F16, name="TQ1", tag="TQ1")
        TK1 = consts.tile([P, C], F16, name="TK1", tag="TK1")
        TQ2 = consts.tile([P, C], F16, name="TQ2", tag="TQ2")
        TK2 = consts.tile([P, C], F16, name="TK2", tag="TK2")
        for t in (TQ1, TK1, TQ2, TK2):
            nc.gpsimd.memset(t[:], 0.0)

        evac_ctr = 0

        def evac(dst, src):
            # Alternate PSUM->SBUF evacuations between DVE and ACT.
            nonlocal evac_ctr
            if evac_ctr % 2 == 0:
                nc.vector.tensor_copy(dst, src)
            else:
                nc.scalar.copy(dst, src)
            evac_ctr += 1

        def stage_a(s):
            """Load x_s (straight + transposed), q/k projections, energy, E."""
            xh = xh_pool.tile([P, CCH, N], F16, name="xh", tag="xh")
            nc.sync.dma_start(xh[:], xh_d.ap()[s].rearrange("(o p) n -> p o n", p=P))

            xfTh = xfT_pool.tile([P, NCH, C], F16, name="xfTh", tag="xfTh")
            nc.sync.dma_start(
                xfTh[:], xth_d.ap()[s].rearrange("(o p) c -> p o c", p=P))
            xfTm = xfT_pool.tile([P, NCH, C], F16, name="xfTm", tag="xfTm")
            nc.sync.dma_start(
                xfTm[:], xtm_d.ap()[s].rearrange("(o p) c -> p o c", p=P))

            # qk^T[d|dk, c] in one PSUM tile: rows 0:64 = q^T, 64:128 = k^T.
            psqk = ps.tile([P, 512], F32, name="psqk", tag="ps")
            nmm = 3 * NCH
            i = 0
            for nb in range(NCH):
                for lhsT, rhs in (
                    (qkmh_sb[:, nb, :], xfTh[:, nb, :]),
                    (qkmm_sb[:, nb, :], xfTh[:, nb, :]),
                    (qkmh_sb[:, nb, :], xfTm[:, nb, :]),
                ):
                    nc.tensor.matmul(psqk[:], lhsT, rhs,
                                     start=(i == 0), stop=(i == nmm - 1))
                    i += 1

            qk_hi = small_pool.tile([P, 512], F16, name="qk_hi", tag="qk_hi")
            nc.vector.tensor_copy(qk_hi[:], psqk[:])
            qk_mid = small_pool.tile([P, 512], F16, name="qk_mid", tag="qk_mid")
            nc.vector.tensor_tensor(
                qk_mid[:], psqk[:], qk_hi[:], mybir.AluOpType.subtract)

            nc.vector.tensor_copy(TQ1[0:64, :], qk_hi[0:64, :])
            nc.vector.tensor_copy(TQ1[64:128, :], qk_mid[0:64, :])
            nc.vector.tensor_copy(TK1[0:64, :], qk_hi[64:128, :])
            nc.vector.tensor_copy(TK1[64:128, :], qk_hi[64:128, :])
            nc.vector.tensor_copy(TQ2[0:64, :], qk_hi[0:64, :])
            nc.vector.tensor_copy(TK2[0:64, :], qk_mid[64:128, :])

            tE = tE_pool.tile([P, CCH, C], F32, name="tE", tag="tE")
            E = tE_pool.tile([P, CCH, C], F16, name="E", tag="E")
            for cb in range(CCH):
                pse = ps.tile([P, 512], F32, name="pse", tag="ps")
                nc.tensor.matmul(pse[:], TQ1[:, cb * P:(cb + 1) * P], TK1[:],
                                 start=True, stop=False)
                nc.tensor.matmul(pse[:], TQ2[:, cb * P:(cb + 1) * P], TK2[:],
                                 start=False, stop=True)
                # sigmoid(x) = 0.5*tanh(0.5x) + 0.5 ; E = exp(0.5*t + 0.5).
                # tanh and exp share one ACT table set (exp_and_others).
                nc.scalar.activation(tE[:, cb, :], pse[:], Tanh, scale=0.5)
            for cb in range(CCH):
                nc.scalar.activation(
                    E[:, cb, :], tE[:, cb, :], Exp, bias=half[:, 0:1], scale=0.5,
                )
            return xh, E

        def stage_b(s, xh, E):
            """Denominators, M1T, final matmul + normalize + store."""
            # rows: [denom; E^T b] = [2, C]
            psr = ps.tile([2, 512], F32, name="psr", tag="ps")
            for icc in range(CCH):
                nc.tensor.matmul(
                    psr[:], ob_sb[:, icc, :], E[:, icc, :],
                    start=(icc == 0), stop=(icc == CCH - 1),
                )
            rows2 = small_pool.tile([2, C], F32, name="rows2", tag="rows2")
            nc.vector.tensor_copy(rows2[:], psr[:])
            nc.vector.reciprocal(rows2[0:1, :], rows2[0:1, :])
            # Transpose [2, C] rows into per-partition columns [128, 2] per j-block.
            psc = ps.tile([P, 2 * CCH], F32, name="psc", tag="ps")
            for jb in range(CCH):
                nc.tensor.transpose(
                    psc[:, 2 * jb:2 * jb + 2],
                    rows2[:, jb * P:(jb + 1) * P],
                    ident[0:2, 0:2],
                )
            scales = small_pool.tile([P, CCH, 2], F32, name="scales", tag="scales")
            nc.vector.tensor_copy(scales[:], psc[:])
            # scales[:, jb, 1] = eb * recip
            nc.vector.tensor_mul(scales[:, :, 1], scales[:, :, 1], scales[:, :, 0])

            # M1T[c', j] = sum_i W[i, c'] E[i, j]
            m1t = m1t_pool.tile([P, CCH, C], F16, name="m1t", tag="m1t")
            for cb in range(CCH):
                psm = ps.tile([P, 512], F32, name="psm", tag="ps")
                for icc in range(CCH):
                    nc.tensor.matmul(
                        psm[:], w_sb[:, icc, cb * P:(cb + 1) * P], E[:, icc, :],
                        start=(icc == 0), stop=(icc == CCH - 1),
                    )
                evac(m1t[:, cb, :], psm[:])

            # out[j, n] = (sum_c' M1T[c', j] x[c', n]) * recip[j] + eb*recip[j]
            o_view = o_d.ap()[s].rearrange("(o p) n -> p o n", p=P)
            for jb in range(CCH):
                pso = [
                    ps.tile([P, 512], F32, name=f"pso{h}", tag="ps")
                    for h in range(2)
                ]
                for cc in range(CCH):
                    for h in range(2):
                        nc.tensor.matmul(
                            pso[h][:],
                            m1t[:, cc, jb * P:(jb + 1) * P],
                            xh[:, cc, h * 512:(h + 1) * 512],
                            start=(cc == 0), stop=(cc == CCH - 1),
                        )
                for h in range(2):
                    ot = out_pool.tile([P, 512], F32, name="ot", tag="ot")
                    if (jb * 2 + h) % 2 == 0:
                        nc.vector.tensor_scalar(
                            ot[:], pso[h][:],
                            scales[:, jb, 0:1], scales[:, jb, 1:2],
                            mybir.AluOpType.mult, mybir.AluOpType.add,
                        )
                    else:
                        nc.scalar.activation(
                            ot[:], pso[h][:], Identity,
                            bias=scales[:, jb, 1:2], scale=scales[:, jb, 0:1],
                        )
                    nc.sync.dma_start(o_view[:, jb, h * 512:(h + 1) * 512], ot[:])

        prev = None
        for s in range(BS):
            cur = stage_a(s)
            if prev is not None:
                stage_b(s - 1, *prev)
            prev = cur
        stage_b(BS - 1, *prev)


_CACHE = {}


def _build():
    if "nc" in _CACHE:
        return _CACHE["nc"]
    nc = bacc.Bacc("TRN2", target_bir_lowering=False, debug=False,
                   num_devices=NCORES)
    xh_d = nc.dram_tensor("x_hi", [BS, C, N], F16, kind="ExternalInput")
    xth_d = nc.dram_tensor("xT_hi", [BS, N, C], F16, kind="ExternalInput")
    xtm_d = nc.dram_tensor("xT_mid", [BS, N, C], F16, kind="ExternalInput")
    qkmh_d = nc.dram_tensor("qkm_hi", [N, 2 * D], F16, kind="ExternalInput")
    qkmm_d = nc.dram_tensor("qkm_mid", [N, 2 * D], F16, kind="ExternalInput")
    w_d = nc.dram_tensor("value_w", [C, C], F16, kind="ExternalInput")
    b_d = nc.dram_tensor("value_b", [C], F32, kind="ExternalInput")
    o_d = nc.dram_tensor("out", [BS, C, N], F32, kind="ExternalOutput")
    with tile.TileContext(nc) as tc:
        _emit(tc, xh_d, xth_d, xtm_d, qkmh_d, qkmm_d, w_d, b_d, o_d)
    nc.compile()
    _CACHE["nc"] = nc
    return nc


def make_in_maps(x, query_m, key_m, value_w, value_b):
    x = np.ascontiguousarray(x.reshape(B, C, N), dtype=np.float32)
    x_hi = x.astype(np.float16)
    xT = np.ascontiguousarray(x.transpose(0, 2, 1))
    xT_hi = xT.astype(np.float16)
    xT_mid = (xT - xT_hi.astype(np.float32)).astype(np.float16)
    qkm = np.concatenate(
        [np.asarray(query_m, np.float32), np.asarray(key_m, np.float32)], axis=1
    )
    qkm_hi = qkm.astype(np.float16)
    qkm_mid = (qkm - qkm_hi.astype(np.float32)).astype(np.float16)
    common = {
        "qkm_hi": np.ascontiguousarray(qkm_hi),
        "qkm_mid": np.ascontiguousarray(qkm_mid),
        "value_w": np.ascontiguousarray(np.asarray(value_w).astype(np.float16)),
        "value_b": np.ascontiguousarray(value_b, dtype=np.float32),
    }
    return [
        {
            "x_hi": np.ascontiguousarray(x_hi[c * BS:(c + 1) * BS]),
            "xT_hi": np.ascontiguousarray(xT_hi[c * BS:(c + 1) * BS]),
            "xT_mid": np.ascontiguousarray(xT_mid[c * BS:(c + 1) * BS]),
            **common,
        }
        for c in range(NCORES)
    ]


def kernel(x, query_m, key_m, value_w, value_b):
    nc = _build()
    in_maps = make_in_maps(x, query_m, key_m, value_w, value_b)
    res = run_bass_kernel_spmd(nc, in_maps, list(range(NCORES)))
    out = np.concatenate([res.results[c]["out"] for c in range(NCORES)], axis=0)
    return out.reshape(B, C, H, W_SP).astype(np.float32)


# revision 11
# speedup vs baseline: 3.1237x; 2.4657x over previous
"""Channel self-attention kernel for Trainium2 (8 NeuronCores, data-parallel over batch).

Per sample (x: [C=512, N=1024], qm/km: [N, 64], W: [C, C], b: [C]):
  q = x @ qm ; k = x @ km                  (contract over n -> needs x^T)
  energy = q @ k^T                         [C, C]
  E = exp(sigmoid(energy))                 (sigmoid via tanh: one ACT table set)
  att = E / colsum(E)                      (softmax over rows, axis -2)
  v = W @ x + b
  out[j, n] = sum_i att[i, j] v[i, n]
Reassociated as:
  M1T[c', j] = sum_i W[i, c'] E[i, j]      (native layouts, no PE transposes)
  unnorm[j, n] = sum_c' M1T[c', j] x[c', n]
  out = unnorm * recip[j] + (E^T b)[j] * recip[j]

Numerics: the host splits x and [qm|km] into fp16 hi/mid pairs (x = hi + mid
to ~21 mantissa bits). fp16 matmuls run at 1 PE cycle/row with exact products
accumulated in fp32 PSUM, and fp16 is XBAR-eligible so x^T comes from
DMA-transpose instead of the (lossy, 12-bit) PE transpose path:
  q/k   : 3-term hi/lo matmuls        -> ~2^-17 relative accuracy
  energy: q/k re-split on device, 2 stacked K=128 matmuls per block
  value : single-term fp16 (contributes ~1e-4 of output absmax)
"""

import numpy as np

import concourse.bass as bass
import concourse.tile as tile
from concourse import bacc, mybir
from concourse.bass_utils import run_bass_kernel_spmd

B, C, N, D = 64, 512, 1024, 64
H = W_SP = 32
NCORES = 8
BS = B // NCORES  # samples per core
P = 128
CCH = C // P  # 4 channel chunks
NCH = N // P  # 8 n chunks

F32 = mybir.dt.float32
F16 = mybir.dt.float16


def _emit(tc, xh_d, xth_d, xtm_d, qkmh_d, qkmm_d, w_d, b_d, o_d, n_samples=BS):
    nc = tc.nc
    from concourse.masks import make_identity
    Tanh = mybir.ActivationFunctionType.Tanh
    Exp = mybir.ActivationFunctionType.Exp
    Identity = mybir.ActivationFunctionType.Identity

    with (
        tc.tile_pool(name="consts", bufs=1) as consts,
        tc.tile_pool(name="xh", bufs=4) as xh_pool,
        tc.tile_pool(name="xfT", bufs=3) as xfT_pool,
        tc.tile_pool(name="tE", bufs=3) as tE_pool,
        tc.tile_pool(name="m1t", bufs=3) as m1t_pool,
        tc.tile_pool(name="small", bufs=3) as small_pool,
        tc.tile_pool(name="outp", bufs=6) as out_pool,
        tc.tile_pool(name="ps", bufs=8, space="PSUM") as ps,
    ):
        ident = consts.tile([P, P], F32, name="ident", tag="ident")
        make_identity(nc, ident[:])

        qkmh_sb = consts.tile([P, NCH, P], F16, name="qkmh_sb", tag="qkmh_sb")
        nc.sync.dma_start(qkmh_sb[:], qkmh_d.ap())
        qkmm_sb = consts.tile([P, NCH, P], F16, name="qkmm_sb", tag="qkmm_sb")
        nc.sync.dma_start(qkmm_sb[:], qkmm_d.ap())
        w_sb = consts.tile([P, CCH, C], F16, name="w_sb", tag="w_sb")
        nc.sync.dma_start(w_sb[:], w_d.ap())

        # ob_sb[:, icc, 0] = 1.0 (denominator), ob_sb[:, icc, 1] = b[icc*128+p]
        b_tmp = consts.tile([P, CCH], F32, name="b_tmp", tag="b_tmp")
        nc.sync.dma_start(b_tmp[:], b_d.ap())
        ob_sb = consts.tile([P, CCH, 2], F16, name="ob_sb", tag="ob_sb")
        nc.gpsimd.memset(ob_sb[:], 1.0)
        nc.vector.tensor_copy(ob_sb[:, :, 1], b_tmp[:])

        half = consts.tile([P, 1], F32, name="half", tag="half")
        nc.gpsimd.memset(half[:], 0.5)

        # Energy operand tiles (persistent): rows 0:64 / 64:128 are rewritten
        # per sample; TQ2/TK2 rows 64:128 stay zero so the second energy
        # matmul contributes only q_hi * k_mid.
        TQ1 = consts.tile([P, C], F16, name="TQ1", tag="TQ1")
        TK1 = consts.tile([P, C], F16, name="TK1", tag="TK1")
        TQ2 = consts.tile([P, C], F16, name="TQ2", tag="TQ2")
        TK2 = consts.tile([P, C], F16, name="TK2", tag="TK2")
        for t in (TQ1, TK1, TQ2, TK2):
            nc.gpsimd.memset(t[:], 0.0)

        evac_ctr = 0

        def evac(dst, src):
            # Alternate PSUM->SBUF evacuations between DVE and ACT.
            nonlocal evac_ctr
            if evac_ctr % 2 == 0:
                nc.vector.tensor_copy(dst, src)
            else:
                nc.scalar.copy(dst, src)
            evac_ctr += 1

        def stage_a(s):
            """Load x_s (straight + transposed), q/k projections, energy, E."""
            xh = xh_pool.tile([P, CCH, N], F16, name="xh", tag="xh")
            nc.sync.dma_start(xh[:], xh_d.ap()[s])

            xfTh = xfT_pool.tile([P, NCH, C], F16, name="xfTh", tag="xfTh")
            nc.sync.dma_start(xfTh[:], xth_d.ap()[s])
            xfTm = xfT_pool.tile([P, NCH, C], F16, name="xfTm", tag="xfTm")
            nc.sync.dma_start(xfTm[:], xtm_d.ap()[s])

            # qk^T[d|dk, c] in one PSUM tile: rows 0:64 = q^T, 64:128 = k^T.
            psqk = ps.tile([P, 512], F32, name="psqk", tag="ps")
            nmm = 3 * NCH
            i = 0
            for nb in range(NCH):
                for lhsT, rhs in (
                    (qkmh_sb[:, nb, :], xfTh[:, nb, :]),
                    (qkmm_sb[:, nb, :], xfTh[:, nb, :]),
                    (qkmh_sb[:, nb, :], xfTm[:, nb, :]),
                ):
                    nc.tensor.matmul(psqk[:], lhsT, rhs,
                                     start=(i == 0), stop=(i == nmm - 1))
                    i += 1

            qk_hi = small_pool.tile([P, 512], F16, name="qk_hi", tag="qk_hi")
            nc.vector.tensor_copy(qk_hi[:], psqk[:])
            qk_mid = small_pool.tile([P, 512], F16, name="qk_mid", tag="qk_mid")
            nc.vector.tensor_tensor(
                qk_mid[:], psqk[:], qk_hi[:], mybir.AluOpType.subtract)

            nc.vector.tensor_copy(TQ1[0:64, :], qk_hi[0:64, :])
            nc.vector.tensor_copy(TQ1[64:128, :], qk_mid[0:64, :])
            nc.vector.tensor_copy(TK1[0:64, :], qk_hi[64:128, :])
            nc.vector.tensor_copy(TK1[64:128, :], qk_hi[64:128, :])
            nc.vector.tensor_copy(TQ2[0:64, :], qk_hi[0:64, :])
            nc.vector.tensor_copy(TK2[0:64, :], qk_mid[64:128, :])

            tE = tE_pool.tile([P, CCH, C], F32, name="tE", tag="tE")
            E = tE_pool.tile([P, CCH, C], F16, name="E", tag="E")
            for cb in range(CCH):
                pse = ps.tile([P, 512], F32, name="pse", tag="ps")
                nc.tensor.matmul(pse[:], TQ1[:, cb * P:(cb + 1) * P], TK1[:],
                                 start=True, stop=False)
                nc.tensor.matmul(pse[:], TQ2[:, cb * P:(cb + 1) * P], TK2[:],
                                 start=False, stop=True)
                # sigmoid(x) = 0.5*tanh(0.5x) + 0.5 ; E = exp(0.5*t + 0.5).
                # tanh and exp share one ACT table set (exp_and_others).
                nc.scalar.activation(tE[:, cb, :], pse[:], Tanh, scale=0.5)
            for cb in range(CCH):
                nc.scalar.activation(
                    E[:, cb, :], tE[:, cb, :], Exp, bias=half[:, 0:1], scale=0.5,
                )
            return xh, E

        def stage_b(s, xh, E):
            """Denominators, M1T, final matmul + normalize + store."""
            # rows: [denom; E^T b] = [2, C]
            psr = ps.tile([2, 512], F32, name="psr", tag="ps")
            for icc in range(CCH):
                nc.tensor.matmul(
                    psr[:], ob_sb[:, icc, :], E[:, icc, :],
                    start=(icc == 0), stop=(icc == CCH - 1),
                )
            rows2 = small_pool.tile([2, C], F32, name="rows2", tag="rows2")
            nc.vector.tensor_copy(rows2[:], psr[:])
            nc.vector.reciprocal(rows2[0:1, :], rows2[0:1, :])
            # Transpose [2, C] rows into per-partition columns [128, 2] per j-block.
            psc = ps.tile([P, 2 * CCH], F32, name="psc", tag="ps")
            for jb in range(CCH):
                nc.tensor.transpose(
                    psc[:, 2 * jb:2 * jb + 2],
                    rows2[:, jb * P:(jb + 1) * P],
                    ident[0:2, 0:2],
                )
            scales = small_pool.tile([P, CCH, 2], F32, name="scales", tag="scales")
            nc.vector.tensor_copy(scales[:], psc[:])
            # scales[:, jb, 1] = eb * recip
            nc.vector.tensor_mul(scales[:, :, 1], scales[:, :, 1], scales[:, :, 0])

            # M1T[c', j] = sum_i W[i, c'] E[i, j]
            m1t = m1t_pool.tile([P, CCH, C], F16, name="m1t", tag="m1t")
            for cb in range(CCH):
                psm = ps.tile([P, 512], F32, name="psm", tag="ps")
                for icc in range(CCH):
                    nc.tensor.matmul(
                        psm[:], w_sb[:, icc, cb * P:(cb + 1) * P], E[:, icc, :],
                        start=(icc == 0), stop=(icc == CCH - 1),
                    )
                evac(m1t[:, cb, :], psm[:])

            # out[j, n] = (sum_c' M1T[c', j] x[c', n]) * recip[j] + eb*recip[j]
            for jb in range(CCH):
                pso = [
                    ps.tile([P, 512], F32, name=f"pso{h}", tag="ps")
                    for h in range(2)
                ]
                for cc in range(CCH):
                    for h in range(2):
                        nc.tensor.matmul(
                            pso[h][:],
                            m1t[:, cc, jb * P:(jb + 1) * P],
                            xh[:, cc, h * 512:(h + 1) * 512],
                            start=(cc == 0), stop=(cc == CCH - 1),
                        )
                ot = out_pool.tile([P, N], F32, name="ot", tag="ot")
                for h in range(2):
                    if h % 2 == 0:
                        nc.vector.tensor_scalar(
                            ot[:, h * 512:(h + 1) * 512], pso[h][:],
                            scales[:, jb, 0:1], scales[:, jb, 1:2],
                            mybir.AluOpType.mult, mybir.AluOpType.add,
                        )
                    else:
                        nc.scalar.activation(
                            ot[:, h * 512:(h + 1) * 512], pso[h][:], Identity,
                            bias=scales[:, jb, 1:2], scale=scales[:, jb, 0:1],
                        )
                nc.sync.dma_start(o_d.ap()[s][jb * P:(jb + 1) * P, :], ot[:])

        prev = None
        for s in range(n_samples):
            cur = stage_a(s)
            if prev is not None:
                stage_b(s - 1, *prev)
            prev = cur
        stage_b(n_samples - 1, *prev)


_CACHE = {}


def _build(n_samples=BS):
    if ("nc", n_samples) in _CACHE:
        return _CACHE[("nc", n_samples)]
    nc = bacc.Bacc("TRN2", target_bir_lowering=False, debug=False,
                   num_devices=NCORES)
    xh_d = nc.dram_tensor("x_hi", [BS, P, CCH, N], F16, kind="ExternalInput")
    xth_d = nc.dram_tensor("xT_hi", [BS, P, NCH, C], F16, kind="ExternalInput")
    xtm_d = nc.dram_tensor("xT_mid", [BS, P, NCH, C], F16, kind="ExternalInput")
    qkmh_d = nc.dram_tensor("qkm_hi", [P, NCH, 2 * D], F16, kind="ExternalInput")
    qkmm_d = nc.dram_tensor("qkm_mid", [P, NCH, 2 * D], F16, kind="ExternalInput")
    w_d = nc.dram_tensor("value_w", [P, CCH, C], F16, kind="ExternalInput")
    b_d = nc.dram_tensor("value_b", [P, CCH], F32, kind="ExternalInput")
    o_d = nc.dram_tensor("out", [BS, C, N], F32, kind="ExternalOutput")
    with tile.TileContext(nc) as tc:
        _emit(tc, xh_d, xth_d, xtm_d, qkmh_d, qkmm_d, w_d, b_d, o_d,
              n_samples=n_samples)
    nc.compile()
    _CACHE[("nc", n_samples)] = nc
    return nc


def _shuf(a):
    """[.., ch*P, free] -> [.., P, ch, free] partition-major SBUF layout."""
    *lead, rows, free = a.shape
    ch = rows // P
    return np.ascontiguousarray(
        a.reshape(*lead, ch, P, free).swapaxes(-3, -2))


def make_in_maps(x, query_m, key_m, value_w, value_b):
    x = np.ascontiguousarray(x.reshape(B, C, N), dtype=np.float32)
    x_hi = x.astype(np.float16)
    xT = np.ascontiguousarray(x.transpose(0, 2, 1))
    xT_hi = xT.astype(np.float16)
    xT_mid = (xT - xT_hi.astype(np.float32)).astype(np.float16)
    qkm = np.concatenate(
        [np.asarray(query_m, np.float32), np.asarray(key_m, np.float32)], axis=1
    )
    qkm_hi = qkm.astype(np.float16)
    qkm_mid = (qkm - qkm_hi.astype(np.float32)).astype(np.float16)
    common = {
        "qkm_hi": _shuf(qkm_hi),
        "qkm_mid": _shuf(qkm_mid),
        "value_w": _shuf(np.asarray(value_w).astype(np.float16)),
        "value_b": _shuf(
            np.asarray(value_b, np.float32)[:, None])[..., 0],
    }
    return [
        {
            "x_hi": _shuf(x_hi[c * BS:(c + 1) * BS]),
            "xT_hi": _shuf(xT_hi[c * BS:(c + 1) * BS]),
            "xT_mid": _shuf(xT_mid[c * BS:(c + 1) * BS]),
            **common,
        }
        for c in range(NCORES)
    ]


def kernel(x, query_m, key_m, value_w, value_b):
    nc = _build()
    in_maps = make_in_maps(x, query_m, key_m, value_w, value_b)
    res = run_bass_kernel_spmd(nc, in_maps, list(range(NCORES)))
    out = np.concatenate([res.results[c]["out"] for c in range(NCORES)], axis=0)
    return out.reshape(B, C, H, W_SP).astype(np.float32)
